# revision 1
# baseline (speedup 1.0000x reference)
"""Grouped self-attention (GQA) Trainium2 kernel.

Problem: B=2, T=2048, D=2048, 16 Q heads / 4 KV heads, head_dim=128,
full RoPE (base 1e6), causal softmax, output projection.

Sharding: 8 cores = 2 batches x 4 KV groups. Core c handles batch c//4,
kv-group c%4 (4 Q heads + 1 KV head). q/k/v projections column-sharded,
o_proj row-sharded; per-core partial outputs are summed on host.

Per-core pipeline (all matmul operands fp16, fp32 PSUM accumulation):
  phase 1: qT/kT/vT = W.T @ x.T (x pre-transposed on host), RoPE fused
           on the PSUM->SBUF eviction, v transposed to [tk, d] blocks
           on the PE.
  phase 2 (S^T layout -- no P transposes needed): per head h, per
           kv-block j: S^T[tk, tq] = kT_j.T @ qT (causal tq >= j*128
           only), diag-block mask on PSUM, exp on ACT writes P^T
           directly to SBUF fp16 (scale=1/sqrt(d) folded in). Softmax
           denominators via ones-vector matmuls over P^T columns;
           reciprocal computed full-rate on DVE after a gpsimd
           partition-broadcast. O^T = V.T @ P^T accumulated over j per
           512-wide tq group; normalization folded into the O^T PSUM
           eviction (tensor_tensor mult with the broadcast reciprocal).
           Y_partial = O^T.T @ Wo_g per 128-row q block, evicted
           ACT/DVE alternating, DMA'd to DRAM.
"""

import os
import sys

import numpy as np

for _p in ("/opt/trn_rl_repo",):
    if _p not in sys.path and os.path.isdir(_p):
        sys.path.insert(0, _p)

import concourse.bass as bass  # noqa: E402
import concourse.mybir as mybir  # noqa: E402
import concourse.tile as tile  # noqa: E402
from concourse import bacc  # noqa: E402
from concourse.bass_utils import run_bass_kernel_spmd  # noqa: E402
from concourse.masks import make_identity  # noqa: E402

B, T, D = 2, 2048, 2048
NH, NKV, HD = 16, 4, 128
G = NKV              # kv groups == cores per batch
AQ = (NH // NKV) * HD  # attention cols per core (4 heads x 128)
NQB = T // 128       # 16 q blocks
KC = D // 128        # 16 contraction chunks for projections
ROPE_BASE = 1000000.0
INV_SQRT_D = 1.0 / float(np.sqrt(HD))

F32 = mybir.dt.float32
FP16 = mybir.dt.float16

# PT row layout: head-major concatenation of per-j strips.
# strip j covers absolute tq in [j*128, 2048), width 2048 - j*128.
PT_OFF = [0] * (NQB + 1)
for _j in range(NQB):
    PT_OFF[_j + 1] = PT_OFF[_j] + (T - _j * 128)
PT_W = PT_OFF[NQB]  # 17408

_CACHE = {}


def _build_nc():
    nc = bacc.Bacc(None, target_bir_lowering=False, debug=False)

    xT_d = nc.dram_tensor("xT", [D, T], FP16, kind="ExternalInput")
    wq_d = nc.dram_tensor("wq", [D, AQ], FP16, kind="ExternalInput")
    wk_d = nc.dram_tensor("wk", [D, HD], FP16, kind="ExternalInput")
    wv_d = nc.dram_tensor("wv", [D, HD], FP16, kind="ExternalInput")
    wo_d = nc.dram_tensor("wo", [AQ, D], FP16, kind="ExternalInput")
    cos_d = nc.dram_tensor("cosT", [HD, T], F32, kind="ExternalInput")
    sin_d = nc.dram_tensor("sinT", [HD, T], F32, kind="ExternalInput")
    mask_d = nc.dram_tensor("mask", [128, 128], F32, kind="ExternalInput")
    y_d = nc.dram_tensor("y", [T, D], F32, kind="ExternalOutput")

    mult = mybir.AluOpType.mult
    add = mybir.AluOpType.add
    Exp = mybir.ActivationFunctionType.Exp

    with tile.TileContext(nc) as tc:
        with (
            tc.tile_pool(name="const", bufs=1) as cpool,
            tc.tile_pool(name="qkv", bufs=1) as qkv_pool,
        ):
            cos_sb = cpool.tile([HD, T], F32, tag="cos")
            sin_sb = cpool.tile([HD, T], F32, tag="sin")
            mask_sb = cpool.tile([128, 128], F32, tag="mask")
            id_fp = cpool.tile([128, 128], FP16, tag="idf")
            ones_sb = cpool.tile([128, 1], FP16, tag="ones")
            nc.sync.dma_start(cos_sb[:], cos_d[:])
            nc.sync.dma_start(sin_sb[:], sin_d[:])
            nc.sync.dma_start(mask_sb[:], mask_d[:])
            make_identity(nc, id_fp[:])
            nc.gpsimd.memset(ones_sb[:], 1.0)

            qT = qkv_pool.tile([128, 4, T], FP16, tag="qT")   # [d, h, t]
            kT = qkv_pool.tile([128, T], FP16, tag="kT")      # [d, t]
            v_sb = qkv_pool.tile([128, T], FP16, tag="v")     # [tk%128, blk*128+d]

            # ---------------- phase 1: projections + rope ----------------
            with (
                tc.tile_pool(name="xt", bufs=1) as xt_pool,
                tc.tile_pool(name="wld", bufs=4) as w_pool,
                tc.tile_pool(name="p1ps", bufs=1, space="PSUM") as pps,
                tc.tile_pool(name="p1vt", bufs=2, space="PSUM") as pvt,
                tc.tile_pool(name="p1tmp", bufs=3) as tmp_pool,
            ):
                xt = xt_pool.tile([128, KC, T], FP16, tag="xt")
                for e in range(KC):
                    nc.sync.dma_start(xt[:, e, :], xT_d[e * 128:(e + 1) * 128, :])

                for s in range(6):
                    if s < 4:
                        src = wq_d[:, s * 128:(s + 1) * 128]
                    elif s == 4:
                        src = wk_d[:, :]
                    else:
                        src = wv_d[:, :]
                    pss = [pps.tile([128, 512], F32, tag=f"proj{t}",
                                    name=f"proj_{s}_{t}")
                           for t in range(4)]
                    for e in range(KC):
                        we = w_pool.tile([128, 128], FP16, tag="w")
                        nc.sync.dma_start(we[:], src[e * 128:(e + 1) * 128, :])
                        for tci in range(4):
                            nc.tensor.matmul(
                                pss[tci][:],
                                we[:],
                                xt[:, e, tci * 512:(tci + 1) * 512],
                                start=(e == 0),
                                stop=(e == KC - 1),
                            )
                    for tci in range(4):
                        tsl = slice(tci * 512, (tci + 1) * 512)
                        ps = pss[tci]
                        if s < 5:
                            dst = qT[:, s, tsl] if s < 4 else kT[:, tsl]
                            t1 = tmp_pool.tile([128, 512], F32, tag="ropetmp")
                            nc.vector.tensor_tensor(t1[:], ps[:], cos_sb[:, tsl], mult)
                            nc.vector.tensor_tensor(
                                dst[0:64, :], ps[64:128, :], sin_sb[0:64, tsl], mult)
                            nc.vector.tensor_tensor(
                                dst[64:128, :], ps[0:64, :], sin_sb[64:128, tsl], mult)
                            nc.vector.tensor_tensor(dst[:], dst[:], t1[:], add)
                        else:
                            # vT chunk [d, t512] -> fp16, then transpose to v blocks
                            vt = tmp_pool.tile([128, 512], FP16, tag="vtmp")
                            nc.scalar.copy(vt[:], ps[:])
                            pst = pvt.tile([128, 512], FP16, tag="vtr")
                            for j4 in range(4):
                                nc.tensor.transpose(
                                    pst[:, j4 * 128:(j4 + 1) * 128],
                                    vt[:, j4 * 128:(j4 + 1) * 128],
                                    id_fp[:],
                                )
                            nc.vector.tensor_copy(v_sb[:, tsl], pst[:])

            # ---------------- phase 2: attention + o-proj ----------------
            with (
                tc.tile_pool(name="wop", bufs=1) as wo_pool,
                tc.tile_pool(name="att", bufs=2) as att_pool,
                tc.tile_pool(name="otp", bufs=1) as ot_pool,
                tc.tile_pool(name="small", bufs=4) as small_pool,
                tc.tile_pool(name="ps_st", bufs=2, space="PSUM") as ps_st_pool,
                tc.tile_pool(name="ps_sum", bufs=2, space="PSUM") as ps_sum_pool,
                tc.tile_pool(name="ps_ot", bufs=2, space="PSUM") as ps_ot_pool,
                tc.tile_pool(name="ps_y", bufs=2, space="PSUM") as ps_y_pool,
            ):
                wo_sb = wo_pool.tile([128, 4, D], FP16, tag="wo")
                for h in range(4):
                    nc.sync.dma_start(
                        wo_sb[:, h, :], wo_d[h * 128:(h + 1) * 128, :])
                OT_all = ot_pool.tile([128, 4, T], FP16, tag="OT")

                cp = 0
                for h in range(4):
                    PTh = att_pool.tile([128, PT_W], FP16, tag="PT")
                    # --- S^T + exp per kv strip j ---
                    for j in range(NQB):
                        W = T - j * 128
                        for c0 in range(0, W, 512):
                            cw = min(512, W - c0)
                            ps_st = ps_st_pool.tile([128, 512], F32, tag="ST")
                            nc.tensor.matmul(
                                ps_st[:, :cw],
                                kT[:, j * 128:(j + 1) * 128],
                                qT[:, h, j * 128 + c0:j * 128 + c0 + cw],
                                start=True,
                                stop=True,
                            )
                            if c0 == 0:
                                nc.vector.tensor_tensor(
                                    ps_st[:, :128], ps_st[:, :128],
                                    mask_sb[:], add)
                            nc.scalar.activation(
                                PTh[:, PT_OFF[j] + c0:PT_OFF[j] + c0 + cw],
                                ps_st[:, :cw],
                                Exp,
                                scale=INV_SQRT_D,
                            )
                    # --- softmax denominators: ones.T @ P^T, per tq chunk ---
                    sums_row = small_pool.tile([1, T], F32, tag="sums")
                    for cc in range(4):
                        t0, t1c = cc * 512, cc * 512 + 512
                        js = [j for j in range(NQB) if j * 128 < t1c]
                        ps1 = ps_sum_pool.tile([1, 512], F32, tag="SUM")
                        for n, j in enumerate(js):
                            tq0 = max(t0, j * 128)
                            nc.tensor.matmul(
                                ps1[:, tq0 - t0:512],
                                ones_sb[:],
                                PTh[:, PT_OFF[j] + tq0 - j * 128:
                                    PT_OFF[j] + t1c - j * 128],
                                start=(n == 0),
                                stop=(n == len(js) - 1),
                            )
                        nc.scalar.copy(sums_row[:, t0:t1c], ps1[:])
                    bc = att_pool.tile([128, T], F32, tag="bc")
                    nc.gpsimd.partition_broadcast(bc[:], sums_row[:])
                    nc.vector.reciprocal(bc[:], bc[:])
                    # --- O^T = V.T @ P^T per 512-wide tq group ---
                    for g in range(4):
                        t0, t1c = g * 512, g * 512 + 512
                        js = [j for j in range(NQB) if j * 128 < t1c]
                        ps_ot = ps_ot_pool.tile([128, 512], F32, tag="OT")
                        for n, j in enumerate(js):
                            tq0 = max(t0, j * 128)
                            nc.tensor.matmul(
                                ps_ot[:, tq0 - t0:512],
                                v_sb[:, j * 128:(j + 1) * 128],
                                PTh[:, PT_OFF[j] + tq0 - j * 128:
                                    PT_OFF[j] + t1c - j * 128],
                                start=(n == 0),
                                stop=(n == len(js) - 1),
                            )
                        nc.vector.tensor_tensor(
                            OT_all[:, h, t0:t1c], ps_ot[:], bc[:, t0:t1c], mult)

                # --- o-proj: Y[tq, n] = sum_h OT_h.T @ Wo_h ---
                for b in range(NQB):
                    for nci in range(4):
                        ps_y = ps_y_pool.tile([128, 512], F32, tag="Y")
                        for h in range(4):
                            nc.tensor.matmul(
                                ps_y[:],
                                OT_all[:, h, b * 128:(b + 1) * 128],
                                wo_sb[:, h, nci * 512:(nci + 1) * 512],
                                start=(h == 0),
                                stop=(h == 3),
                            )
                        y_sb = att_pool.tile([128, 512], F32, tag="ysb")
                        if cp % 2 == 0:
                            nc.scalar.copy(y_sb[:], ps_y[:])
                        else:
                            nc.vector.tensor_copy(y_sb[:], ps_y[:])
                        cp += 1
                        nc.sync.dma_start(
                            y_d[b * 128:(b + 1) * 128, nci * 512:(nci + 1) * 512],
                            y_sb[:])

    nc.compile()
    return nc


def _rope_tables():
    # match reference float32 arithmetic exactly
    pos = np.arange(T, dtype=np.float32)
    inv_freq = (1.0 / (ROPE_BASE ** (np.arange(0, HD, 2, dtype=np.float32) / HD))).astype(np.float32)
    ang = pos[:, None] * inv_freq[None, :]            # [T, 64]
    cos = np.cos(ang).astype(np.float32)
    sin = np.sin(ang).astype(np.float32)
    cosT = np.ascontiguousarray(np.concatenate([cos, cos], 1).T)   # [128, T]
    sinT = np.ascontiguousarray(np.concatenate([-sin, sin], 1).T)  # rotate_half sign
    return cosT, sinT


def kernel(x, Wq, bq, Wk, bk, Wv, bv, Wo, bo, **_ignored):
    x = np.asarray(x, dtype=np.float32)
    Wq = np.asarray(Wq, dtype=np.float32)
    Wk = np.asarray(Wk, dtype=np.float32)
    Wv = np.asarray(Wv, dtype=np.float32)
    Wo = np.asarray(Wo, dtype=np.float32)
    bo = np.asarray(bo, dtype=np.float32)

    if "nc" not in _CACHE:
        _CACHE["nc"] = _build_nc()
    nc = _CACHE["nc"]

    cosT, sinT = _rope_tables()
    # S^T layout: mask[tk, tq] allows tk <= tq within the diagonal block
    triu = np.triu(np.ones((128, 128), dtype=bool))
    mask = np.where(triu, 0.0, -1e9).astype(np.float32)

    in_maps = []
    for c in range(8):
        b, g = c // G, c % G
        in_maps.append({
            "xT": np.ascontiguousarray(x[b].T.astype(np.float16)),
            "wq": np.ascontiguousarray(Wq[:, g * AQ:(g + 1) * AQ].astype(np.float16)),
            "wk": np.ascontiguousarray(Wk[:, g * HD:(g + 1) * HD].astype(np.float16)),
            "wv": np.ascontiguousarray(Wv[:, g * HD:(g + 1) * HD].astype(np.float16)),
            "wo": np.ascontiguousarray(Wo[g * AQ:(g + 1) * AQ, :].astype(np.float16)),
            "cosT": cosT,
            "sinT": sinT,
            "mask": mask,
        })

    res = run_bass_kernel_spmd(
        nc, in_maps, list(range(8)),
        trace=bool(os.environ.get("KERNEL_TRACE")),
        tmpdir=os.environ.get("KERNEL_TRACE_DIR") or None,
    )
    _CACHE["last_results"] = res

    out = np.zeros((B, T, D), dtype=np.float32)
    for b in range(B):
        acc = np.zeros((T, D), dtype=np.float32)
        for g in range(G):
            acc += res.results[b * G + g]["y"]
        out[b] = acc + bo[None, :]
    return out



# revision 2
# speedup vs baseline: 1.1335x; 1.1335x over previous
"""Grouped self-attention (GQA) Trainium2 kernel, v2.

Problem: B=2, T=2048, D=2048, 16 Q heads / 4 KV heads, head_dim=128,
full RoPE (base 1e6), causal softmax, output projection.

Sharding: 8 cores = 2 batches x 4 KV groups. Core c handles batch c//4,
kv-group c%4 (4 Q heads + 1 KV head). q/k/v projections column-sharded,
o_proj row-sharded; per-core partial outputs are summed on host.

v2 structure (vs v1): weights preloaded to SBUF with x/w DMA issue
interleaved so the PE starts ~2us in; projections run tci-outer/e-inner
with a double-buffered PSUM tile (no eviction stalls); k/v projected
first; attention is tq-group-major (4 groups of 512) with the o-proj
emitted per group so its PE work and the y DMA-out spread through the
kernel instead of forming a 127us serial tail; softmax reciprocal is
taken on [1,512] BEFORE the partition broadcast (v1 reciprocal'd the
broadcast 128x copy: 4x13us DVE); softmax-denominator ones-matmuls are
interleaved into the score-strip loop to hide the exp latency; y is
written fp16 on a second DMA queue.
"""

import os
import sys

import numpy as np

for _p in ("/opt/trn_rl_repo",):
    if _p not in sys.path and os.path.isdir(_p):
        sys.path.insert(0, _p)

import concourse.bass as bass  # noqa: E402
import concourse.mybir as mybir  # noqa: E402
import concourse.tile as tile  # noqa: E402
from concourse import bacc  # noqa: E402
from concourse.bass_utils import run_bass_kernel_spmd  # noqa: E402
from concourse.masks import make_identity  # noqa: E402

B, T, D = 2, 2048, 2048
NH, NKV, HD = 16, 4, 128
G = NKV              # kv groups == cores per batch
AQ = (NH // NKV) * HD  # attention cols per core (4 heads x 128)
KC = D // 128        # 16 contraction chunks for projections
ROPE_BASE = 1000000.0
INV_SQRT_D = 1.0 / float(np.sqrt(HD))

F32 = mybir.dt.float32
FP16 = mybir.dt.float16

PT_MAX = 512 * 13 + 768  # widest per-(g,h) P^T row (g=3): 7424


def _strips(g):
    """Score strips for tq group g (cols [512g, 512g+512)).

    Returns [(j, off, w)]: kv block j covers group columns
    [512 - w, 512) absolutely starting at max(512g, 128j); off is the
    strip's offset in the packed per-(g,h) P^T buffer.
    """
    out = []
    off = 0
    for j in range(4 * g + 4):
        w = 512 - max(0, 128 * j - 512 * g)
        out.append((j, off, w))
        off += w
    return out


_CACHE = {}


def _build_nc():
    nc = bacc.Bacc(None, target_bir_lowering=False, debug=False)

    xT_d = nc.dram_tensor("xT", [D, T], FP16, kind="ExternalInput")
    wq_d = nc.dram_tensor("wq", [D, AQ], FP16, kind="ExternalInput")
    wk_d = nc.dram_tensor("wk", [D, HD], FP16, kind="ExternalInput")
    wv_d = nc.dram_tensor("wv", [D, HD], FP16, kind="ExternalInput")
    wo_d = nc.dram_tensor("wo", [AQ, D], FP16, kind="ExternalInput")
    cos_d = nc.dram_tensor("cosT", [HD, T], F32, kind="ExternalInput")
    sin_d = nc.dram_tensor("sinT", [HD, T], F32, kind="ExternalInput")
    mask_d = nc.dram_tensor("mask", [128, 128], F32, kind="ExternalInput")
    y_d = nc.dram_tensor("y", [T, D], FP16, kind="ExternalOutput")

    mult = mybir.AluOpType.mult
    add = mybir.AluOpType.add
    Exp = mybir.ActivationFunctionType.Exp

    with tile.TileContext(nc) as tc:
        with (
            tc.tile_pool(name="const", bufs=1) as cpool,
            tc.tile_pool(name="qkv", bufs=1) as qkv_pool,
            tc.tile_pool(name="xw", bufs=1) as xw_pool,
            tc.tile_pool(name="ptmp", bufs=2) as tmp_pool,
            tc.tile_pool(name="pp", bufs=2, space="PSUM") as pp_pool,
        ):
            cos_sb = cpool.tile([HD, T], F32, tag="cos")
            sin_sb = cpool.tile([HD, T], F32, tag="sin")
            mask_sb = cpool.tile([128, 128], F32, tag="mask")
            id_fp = cpool.tile([128, 128], FP16, tag="idf")
            ones_sb = cpool.tile([128, 1], FP16, tag="ones")
            wo_sb = cpool.tile([128, 4, D], FP16, tag="wo")

            # x + projection weights interleaved on the sync queue; the
            # rope tables / mask / o-proj weights go on the gpsimd queue
            # so neither stream delays the other.
            xt = xw_pool.tile([128, KC, T], FP16, tag="xt")
            w_all = xw_pool.tile([128, KC, 768], FP16, tag="w")
            nc.gpsimd.dma_start(mask_sb[:], mask_d[:])
            for e in range(KC):
                r = slice(e * 128, (e + 1) * 128)
                nc.sync.dma_start(xt[:, e, :], xT_d[r, :])
                nc.sync.dma_start(w_all[:, e, 512:640], wk_d[r, :])
                nc.sync.dma_start(w_all[:, e, 640:768], wv_d[r, :])
                nc.sync.dma_start(w_all[:, e, 0:512], wq_d[r, :])
            nc.gpsimd.dma_start(cos_sb[:], cos_d[:])
            nc.gpsimd.dma_start(sin_sb[:], sin_d[:])
            for h in range(4):
                nc.gpsimd.dma_start(
                    wo_sb[:, h, :], wo_d[h * 128:(h + 1) * 128, :])
            make_identity(nc, id_fp[:])
            nc.gpsimd.memset(ones_sb[:], 1.0)

            qT = qkv_pool.tile([128, 4, T], FP16, tag="qT")   # [d, h, t]
            kT = qkv_pool.tile([128, T], FP16, tag="kT")      # [d, t]
            v_sb = qkv_pool.tile([128, T], FP16, tag="v")     # [tk%128, blk*128+d]

            def proj_pass(s, tci, pvt_pool=None):
                """Project out-block s (0-3: q heads, 4: k, 5: v) for
                t columns [512*tci, 512*tci+512); rope fused on evict."""
                tsl = slice(tci * 512, (tci + 1) * 512)
                ws = slice(s * 128, (s + 1) * 128)
                ps = pp_pool.tile([128, 512], F32, tag="proj",
                                  name=f"proj_{s}_{tci}")
                for e in range(KC):
                    nc.tensor.matmul(
                        ps[:],
                        w_all[:, e, ws],
                        xt[:, e, tsl],
                        start=(e == 0),
                        stop=(e == KC - 1),
                    )
                if s < 5:
                    dst = qT[:, s, tsl] if s < 4 else kT[:, tsl]
                    t1 = tmp_pool.tile([128, 512], F32, tag="ropetmp")
                    nc.vector.tensor_tensor(t1[:], ps[:], cos_sb[:, tsl], mult)
                    nc.vector.tensor_tensor(
                        dst[0:64, :], ps[64:128, :], sin_sb[0:64, tsl], mult)
                    nc.vector.tensor_tensor(
                        dst[64:128, :], ps[0:64, :], sin_sb[64:128, tsl], mult)
                    nc.vector.tensor_tensor(dst[:], dst[:], t1[:], add)
                else:
                    vt = tmp_pool.tile([128, 512], FP16, tag="vtmp")
                    nc.scalar.copy(vt[:], ps[:])
                    pst = pvt_pool.tile([128, 512], FP16, tag="vtr")
                    for j4 in range(4):
                        nc.tensor.transpose(
                            pst[:, j4 * 128:(j4 + 1) * 128],
                            vt[:, j4 * 128:(j4 + 1) * 128],
                            id_fp[:],
                        )
                    nc.vector.tensor_copy(v_sb[:, tsl], pst[:])

            # ---- k, v first (attention needs them in full), then q tci0
            with tc.tile_pool(name="pvt", bufs=2, space="PSUM") as pvt_pool:
                for tci in range(4):
                    proj_pass(4, tci)
                for tci in range(4):
                    proj_pass(5, tci, pvt_pool)
            for h in range(4):
                proj_pass(h, 0)

            # ---- attention, tq-group-major, o-proj per group ----
            with (
                tc.tile_pool(name="att", bufs=2) as att_pool,
                tc.tile_pool(name="small", bufs=2) as small_pool,
                tc.tile_pool(name="yev", bufs=4) as yev_pool,
                tc.tile_pool(name="ps_st", bufs=2, space="PSUM") as st_pool,
                tc.tile_pool(name="ps_sum", bufs=1, space="PSUM") as sum_pool,
                tc.tile_pool(name="ps_ot", bufs=1, space="PSUM") as ot_pool,
                tc.tile_pool(name="ps_y", bufs=2, space="PSUM") as y_pool,
            ):
                cp = 0
                for g in range(4):
                    sl = _strips(g)
                    gts = slice(512 * g, 512 * g + 512)
                    OTg = att_pool.tile([128, 4, 512], FP16, tag="OTg")
                    for h in range(4):
                        PT = att_pool.tile([128, PT_MAX], FP16, tag="PT")
                        ps1 = sum_pool.tile([1, 512], F32, tag="SUM")
                        # score strips, exp eviction, with the
                        # denominator ones-matmul for strip i emitted
                        # after strip i+1's score matmul (hides exp lag)
                        for idx, (j, off, w) in enumerate(sl):
                            ps = st_pool.tile([128, 512], F32, tag="ST")
                            tq0 = max(512 * g, 128 * j)
                            nc.tensor.matmul(
                                ps[:, :w],
                                kT[:, j * 128:(j + 1) * 128],
                                qT[:, h, tq0:512 * g + 512],
                                start=True,
                                stop=True,
                            )
                            if j >= 4 * g:
                                nc.vector.tensor_tensor(
                                    ps[:, :128], ps[:, :128], mask_sb[:], add)
                            nc.scalar.activation(
                                PT[:, off:off + w], ps[:, :w], Exp,
                                scale=INV_SQRT_D)
                            if idx > 0:
                                pj, poff, pw = sl[idx - 1]
                                nc.tensor.matmul(
                                    ps1[:, 512 - pw:512],
                                    ones_sb[:],
                                    PT[:, poff:poff + pw],
                                    start=(idx == 1),
                                    stop=False,
                                    skip_group_check=True,
                                )
                        lj, loff, lw = sl[-1]
                        nc.tensor.matmul(
                            ps1[:, 512 - lw:512],
                            ones_sb[:],
                            PT[:, loff:loff + lw],
                            start=(len(sl) == 1),
                            stop=True,
                            skip_group_check=True,
                        )
                        sums_sb = small_pool.tile([1, 512], F32, tag="sums")
                        nc.scalar.copy(sums_sb[:], ps1[:])
                        rcp = small_pool.tile([1, 512], F32, tag="rcp")
                        nc.vector.reciprocal(rcp[:], sums_sb[:])
                        bc = small_pool.tile([128, 512], F32, tag="bc")
                        nc.gpsimd.partition_broadcast(bc[:], rcp[:])
                        # O^T accumulation over strips
                        pso = ot_pool.tile([128, 512], F32, tag="OT")
                        for idx, (j, off, w) in enumerate(sl):
                            nc.tensor.matmul(
                                pso[:, 512 - w:512],
                                v_sb[:, j * 128:(j + 1) * 128],
                                PT[:, off:off + w],
                                start=(idx == 0),
                                stop=(idx == len(sl) - 1),
                            )
                        nc.vector.tensor_tensor(
                            OTg[:, h, :], pso[:], bc[:], mult)
                        # prefetch next tq group's q projection
                        if g < 3:
                            proj_pass(h, g + 1)
                    # o-proj for this tq group
                    for tb in range(4):
                        for nci in range(4):
                            psy = y_pool.tile([128, 512], F32, tag="Y")
                            for h in range(4):
                                nc.tensor.matmul(
                                    psy[:],
                                    OTg[:, h, tb * 128:(tb + 1) * 128],
                                    wo_sb[:, h, nci * 512:(nci + 1) * 512],
                                    start=(h == 0),
                                    stop=(h == 3),
                                )
                            ysb = yev_pool.tile([128, 512], FP16, tag="ysb")
                            if cp % 2 == 0:
                                nc.scalar.copy(ysb[:], psy[:])
                            else:
                                nc.vector.tensor_copy(ysb[:], psy[:])
                            cp += 1
                            nc.gpsimd.dma_start(
                                y_d[512 * g + tb * 128:512 * g + tb * 128 + 128,
                                    nci * 512:(nci + 1) * 512],
                                ysb[:])

    nc.compile()
    return nc


def _rope_tables():
    # match reference float32 arithmetic exactly
    pos = np.arange(T, dtype=np.float32)
    inv_freq = (1.0 / (ROPE_BASE ** (np.arange(0, HD, 2, dtype=np.float32) / HD))).astype(np.float32)
    ang = pos[:, None] * inv_freq[None, :]            # [T, 64]
    cos = np.cos(ang).astype(np.float32)
    sin = np.sin(ang).astype(np.float32)
    cosT = np.ascontiguousarray(np.concatenate([cos, cos], 1).T)   # [128, T]
    sinT = np.ascontiguousarray(np.concatenate([-sin, sin], 1).T)  # rotate_half sign
    return cosT, sinT


def kernel(x, Wq, bq, Wk, bk, Wv, bv, Wo, bo, **_ignored):
    x = np.asarray(x, dtype=np.float32)
    Wq = np.asarray(Wq, dtype=np.float32)
    Wk = np.asarray(Wk, dtype=np.float32)
    Wv = np.asarray(Wv, dtype=np.float32)
    Wo = np.asarray(Wo, dtype=np.float32)
    bo = np.asarray(bo, dtype=np.float32)

    if "nc" not in _CACHE:
        _CACHE["nc"] = _build_nc()
    nc = _CACHE["nc"]

    cosT, sinT = _rope_tables()
    # S^T layout: mask[tk, tq] allows tk <= tq within the diagonal block
    triu = np.triu(np.ones((128, 128), dtype=bool))
    mask = np.where(triu, 0.0, -1e9).astype(np.float32)

    in_maps = []
    for c in range(8):
        b, g = c // G, c % G
        in_maps.append({
            "xT": np.ascontiguousarray(x[b].T.astype(np.float16)),
            "wq": np.ascontiguousarray(Wq[:, g * AQ:(g + 1) * AQ].astype(np.float16)),
            "wk": np.ascontiguousarray(Wk[:, g * HD:(g + 1) * HD].astype(np.float16)),
            "wv": np.ascontiguousarray(Wv[:, g * HD:(g + 1) * HD].astype(np.float16)),
            "wo": np.ascontiguousarray(Wo[g * AQ:(g + 1) * AQ, :].astype(np.float16)),
            "cosT": cosT,
            "sinT": sinT,
            "mask": mask,
        })

    res = run_bass_kernel_spmd(
        nc, in_maps, list(range(8)),
        trace=bool(os.environ.get("KERNEL_TRACE")),
        tmpdir=os.environ.get("KERNEL_TRACE_DIR") or None,
    )
    _CACHE["last_results"] = res

    out = np.zeros((B, T, D), dtype=np.float32)
    for b in range(B):
        acc = np.zeros((T, D), dtype=np.float32)
        for g in range(G):
            acc += res.results[b * G + g]["y"].astype(np.float32)
        out[b] = acc + bo[None, :]
    return out


# revision 3
# speedup vs baseline: 1.6676x; 1.4711x over previous
"""Grouped self-attention (GQA) Trainium2 kernel, v3.

Problem: B=2, T=2048, D=2048, 16 Q heads / 4 KV heads, head_dim=128,
full RoPE (base 1e6), causal softmax, output projection.

Sharding: 8 cores = 2 batches x 4 KV groups. Core c handles batch c//4,
kv-group c%4 (4 Q heads + 1 KV head). q/k/v projections column-sharded,
o_proj row-sharded; per-core partial outputs are summed on host.

v3 (vs v2):
- x and qkv weights are packed on host into the exact SBUF layout so
  they stream as a handful of large DMAs (geometric chunks, x and w
  interleaved on the sync queue; rope tables / wo on the gpsimd queue).
- k/v projections run e-outer with 8 PSUM accumulators so the PE
  consumes x chunks as they arrive (no 24us x-load bubble), and the PE
  p-state ramps once and stays at 2.4GHz.
- softmax denominators come from a DVE strip-accumulation R(+)=P^T
  (one [1,512] ones-matmul per (g,h) instead of per strip: -144
  matmuls, -40us PE).
- reciprocal: partition-broadcast first, then reciprocal_approx_fast
  on [128,512] (full-rate custom DVE op; v2 burned 4us/call on a
  1-lane [1,512] exact reciprocal).
- optional ldweights=False on matmuls whose stationary was just loaded
  by the previous matmul (k/v e-outer groups, q-projection head-pairs).
"""

import os
import sys

import numpy as np

for _p in ("/opt/trn_rl_repo",):
    if _p not in sys.path and os.path.isdir(_p):
        sys.path.insert(0, _p)

import concourse.bass as bass  # noqa: E402
import concourse.mybir as mybir  # noqa: E402
import concourse.tile as tile  # noqa: E402
from concourse import bacc  # noqa: E402
from concourse.bass_utils import run_bass_kernel_spmd  # noqa: E402
from concourse.masks import make_identity  # noqa: E402

B, T, D = 2, 2048, 2048
NH, NKV, HD = 16, 4, 128
G = NKV              # kv groups == cores per batch
AQ = (NH // NKV) * HD  # attention cols per core (4 heads x 128)
KC = D // 128        # 16 contraction chunks for projections
ROPE_BASE = 1000000.0
INV_SQRT_D = 1.0 / float(np.sqrt(HD))

F32 = mybir.dt.float32
FP16 = mybir.dt.float16

PT_MAX = 512 * 13 + 768  # widest per-(g,h) P^T row (g=3): 7424

LDW_SKIP = False  # set True once the garbage-stationary probe passes


def _strips(g):
    """Score strips for tq group g (cols [512g, 512g+512)).

    Returns [(j, off, w)]: kv block j contributes group columns
    [512-w, 512) (absolute tq from max(512g, 128j)); off is the strip's
    offset in the packed per-(g,h) P^T buffer.
    """
    out = []
    off = 0
    for j in range(4 * g + 4):
        w = 512 - max(0, 128 * j - 512 * g)
        out.append((j, off, w))
        off += w
    return out


_CACHE = {}


def _build_nc():
    nc = bacc.Bacc(None, target_bir_lowering=False, debug=False)

    # host-packed inputs (see kernel() for layouts)
    xp_d = nc.dram_tensor("xp", [128, KC, T], FP16, kind="ExternalInput")
    wp_d = nc.dram_tensor("wp", [128, KC, 768], FP16, kind="ExternalInput")
    wo_d = nc.dram_tensor("wo", [128, 4, D], FP16, kind="ExternalInput")
    cos_d = nc.dram_tensor("cosT", [HD, T], F32, kind="ExternalInput")
    sin_d = nc.dram_tensor("sinT", [HD, T], F32, kind="ExternalInput")
    mask_d = nc.dram_tensor("mask", [128, 128], F32, kind="ExternalInput")
    y_d = nc.dram_tensor("y", [T, D], FP16, kind="ExternalOutput")

    mult = mybir.AluOpType.mult
    add = mybir.AluOpType.add
    Exp = mybir.ActivationFunctionType.Exp

    def mm(out, lhsT, rhs, start, stop, skip_ldw=False, **kw):
        inst = nc.tensor.matmul(out, lhsT, rhs, start=start, stop=stop, **kw)
        if skip_ldw and LDW_SKIP:
            inst.ldweights = False
        return inst

    with tile.TileContext(nc) as tc:
        with (
            tc.tile_pool(name="const", bufs=1) as cpool,
            tc.tile_pool(name="qkv", bufs=1) as qkv_pool,
            tc.tile_pool(name="xw", bufs=1) as xw_pool,
            tc.tile_pool(name="ptmp", bufs=2) as tmp_pool,
        ):
            cos_sb = cpool.tile([HD, T], F32, tag="cos")
            sin_sb = cpool.tile([HD, T], F32, tag="sin")
            mask_sb = cpool.tile([128, 128], F32, tag="mask")
            id_fp = cpool.tile([128, 128], FP16, tag="idf")
            ones_sb = cpool.tile([128, 1], FP16, tag="ones")
            wo_sb = cpool.tile([128, 4, D], FP16, tag="wo")

            xt = xw_pool.tile([128, KC, T], FP16, tag="xt")
            w_all = xw_pool.tile([128, KC, 768], FP16, tag="w")
            # x/w stream in geometric chunks, interleaved, sync queue;
            # everything else on the gpsimd queue.
            nc.gpsimd.dma_start(mask_sb[:], mask_d[:])
            for e0, e1 in ((0, 1), (1, 2), (2, 4), (4, 8), (8, 16)):
                nc.sync.dma_start(w_all[:, e0:e1, :], wp_d[:, e0:e1, :])
                nc.sync.dma_start(xt[:, e0:e1, :], xp_d[:, e0:e1, :])
            nc.gpsimd.dma_start(cos_sb[:], cos_d[:])
            nc.gpsimd.dma_start(sin_sb[:], sin_d[:])
            nc.gpsimd.dma_start(wo_sb[:], wo_d[:])
            make_identity(nc, id_fp[:])
            nc.gpsimd.memset(ones_sb[:], 1.0)

            qT = qkv_pool.tile([128, 4, T], FP16, tag="qT")   # [d, h, t]
            kT = qkv_pool.tile([128, T], FP16, tag="kT")      # [d, t]
            v_sb = qkv_pool.tile([128, T], FP16, tag="v")     # [tk%128, blk*128+d]
            vT_sb = qkv_pool.tile([128, T], FP16, tag="vT")   # [d, t] pre-transpose

            def rope_evict(ps, dst, tsl):
                t1 = tmp_pool.tile([128, 512], F32, tag="ropetmp")
                nc.vector.tensor_tensor(t1[:], ps[:], cos_sb[:, tsl], mult)
                nc.vector.tensor_tensor(
                    dst[0:64, :], ps[64:128, :], sin_sb[0:64, tsl], mult)
                nc.vector.tensor_tensor(
                    dst[64:128, :], ps[0:64, :], sin_sb[64:128, tsl], mult)
                nc.vector.tensor_tensor(dst[:], dst[:], t1[:], add)

            # ---- phase 1a: k+v projections, e-outer (DMA-streamed) ----
            with tc.tile_pool(name="ppkv", bufs=1, space="PSUM") as ppkv:
                psk = [ppkv.tile([128, 512], F32, tag=f"k{t}",
                                 name=f"psk{t}") for t in range(4)]
                psv = [ppkv.tile([128, 512], F32, tag=f"v{t}",
                                 name=f"psv{t}") for t in range(4)]
                for e in range(KC):
                    for tci in range(4):
                        mm(psk[tci][:], w_all[:, e, 512:640],
                           xt[:, e, tci * 512:(tci + 1) * 512],
                           start=(e == 0), stop=(e == KC - 1),
                           skip_ldw=(tci > 0))
                    for tci in range(4):
                        mm(psv[tci][:], w_all[:, e, 640:768],
                           xt[:, e, tci * 512:(tci + 1) * 512],
                           start=(e == 0), stop=(e == KC - 1),
                           skip_ldw=(tci > 0))
                for tci in range(4):
                    tsl = slice(tci * 512, (tci + 1) * 512)
                    rope_evict(psk[tci], kT[:, tsl], tsl)
                    nc.scalar.copy(vT_sb[:, tsl], psv[tci][:])

            # ---- phase 1b: v transpose + q tci0, pairs sharing LDW ----
            def q_pass_pair(ha, tci):
                """Project q heads ha, ha+1 for t cols [512*tci, ...)."""
                tsl = slice(tci * 512, (tci + 1) * 512)
                pa = pq_pool.tile([128, 512], F32, tag="qa",
                                  name=f"q{ha}_{tci}")
                pb = pq_pool.tile([128, 512], F32, tag="qb",
                                  name=f"q{ha + 1}_{tci}")
                for e in range(KC):
                    mm(pa[:], w_all[:, e, ha * 128:(ha + 1) * 128],
                       xt[:, e, tsl], start=(e == 0), stop=(e == KC - 1))
                    mm(pb[:], w_all[:, e, (ha + 1) * 128:(ha + 2) * 128],
                       xt[:, e, tsl], start=(e == 0), stop=(e == KC - 1))
                rope_evict(pa, qT[:, ha, tsl], tsl)
                rope_evict(pb, qT[:, ha + 1, tsl], tsl)

            with tc.tile_pool(name="pq", bufs=1, space="PSUM") as pq_pool:
                with tc.tile_pool(name="pvt", bufs=2, space="PSUM") as pvt:
                    for tci in range(4):
                        tsl = slice(tci * 512, (tci + 1) * 512)
                        pst = pvt.tile([128, 512], FP16, tag="vtr")
                        for j4 in range(4):
                            nc.tensor.transpose(
                                pst[:, j4 * 128:(j4 + 1) * 128],
                                vT_sb[:, tci * 512 + j4 * 128:
                                      tci * 512 + (j4 + 1) * 128],
                                id_fp[:],
                            )
                        nc.vector.tensor_copy(v_sb[:, tsl], pst[:])
                        if tci % 2 == 1:
                            q_pass_pair(tci - 1, 0)

                # ---- attention, tq-group-major, o-proj per group ----
                with (
                    tc.tile_pool(name="att", bufs=2) as att_pool,
                    tc.tile_pool(name="small", bufs=2) as small_pool,
                    tc.tile_pool(name="yev", bufs=4) as yev_pool,
                    tc.tile_pool(name="ps_st", bufs=2, space="PSUM") as st_pool,
                    tc.tile_pool(name="ps_sum", bufs=1, space="PSUM") as sum_pool,
                    tc.tile_pool(name="ps_ot", bufs=1, space="PSUM") as ot_pool,
                    tc.tile_pool(name="ps_y", bufs=2, space="PSUM") as y_pool,
                ):
                    cp = 0
                    for g in range(4):
                        sl = _strips(g)
                        OTg = att_pool.tile([128, 4, 512], FP16, tag="OTg")
                        for h in range(4):
                            PT = att_pool.tile([128, PT_MAX], FP16, tag="PT")
                            R = att_pool.tile([128, 512], FP16, tag="R")
                            for idx, (j, off, w) in enumerate(sl):
                                ps = st_pool.tile([128, 512], F32, tag="ST")
                                tq0 = max(512 * g, 128 * j)
                                mm(ps[:, :w],
                                   kT[:, j * 128:(j + 1) * 128],
                                   qT[:, h, tq0:512 * g + 512],
                                   start=True, stop=True)
                                if j >= 4 * g:
                                    nc.vector.tensor_tensor(
                                        ps[:, :128], ps[:, :128],
                                        mask_sb[:], add)
                                nc.scalar.activation(
                                    PT[:, off:off + w], ps[:, :w], Exp,
                                    scale=INV_SQRT_D)
                                if idx == 0:
                                    nc.vector.tensor_copy(
                                        R[:], PT[:, 0:512])
                                else:
                                    nc.vector.tensor_tensor(
                                        R[:, 512 - w:], R[:, 512 - w:],
                                        PT[:, off:off + w], add)
                            # O^T accumulation over strips (PE busy while
                            # the R chain and denominator chain finish)
                            pso = ot_pool.tile([128, 512], F32, tag="OT")
                            for idx, (j, off, w) in enumerate(sl):
                                mm(pso[:, 512 - w:512],
                                   v_sb[:, j * 128:(j + 1) * 128],
                                   PT[:, off:off + w],
                                   start=(idx == 0),
                                   stop=(idx == len(sl) - 1))
                            ps1 = sum_pool.tile([1, 512], F32, tag="SUM")
                            mm(ps1[:], ones_sb[:], R[:], start=True, stop=True)
                            s_sb = small_pool.tile([1, 512], F32, tag="s")
                            nc.scalar.copy(s_sb[:], ps1[:])
                            bc = small_pool.tile([128, 512], F32, tag="bc")
                            nc.gpsimd.partition_broadcast(bc[:], s_sb[:])
                            rcp = small_pool.tile([128, 512], F32, tag="rcp")
                            nc.vector.reciprocal_approx_fast(rcp[:], bc[:])
                            nc.vector.tensor_tensor(
                                OTg[:, h, :], pso[:], rcp[:], mult)
                            # prefetch next tq group's q projections
                            if g < 3 and h % 2 == 1:
                                q_pass_pair(h - 1, g + 1)
                        # o-proj for this tq group
                        for tb in range(4):
                            for nci in range(4):
                                psy = y_pool.tile([128, 512], F32, tag="Y")
                                for h in range(4):
                                    mm(psy[:],
                                       OTg[:, h, tb * 128:(tb + 1) * 128],
                                       wo_sb[:, h, nci * 512:(nci + 1) * 512],
                                       start=(h == 0), stop=(h == 3))
                                ysb = yev_pool.tile([128, 512], FP16,
                                                    tag="ysb")
                                if cp % 2 == 0:
                                    nc.scalar.copy(ysb[:], psy[:])
                                else:
                                    nc.vector.tensor_copy(ysb[:], psy[:])
                                cp += 1
                                nc.gpsimd.dma_start(
                                    y_d[512 * g + tb * 128:
                                        512 * g + tb * 128 + 128,
                                        nci * 512:(nci + 1) * 512],
                                    ysb[:])

    nc.compile()
    return nc


def _rope_tables():
    # match reference float32 arithmetic exactly
    pos = np.arange(T, dtype=np.float32)
    inv_freq = (1.0 / (ROPE_BASE ** (np.arange(0, HD, 2, dtype=np.float32) / HD))).astype(np.float32)
    ang = pos[:, None] * inv_freq[None, :]            # [T, 64]
    cos = np.cos(ang).astype(np.float32)
    sin = np.sin(ang).astype(np.float32)
    cosT = np.ascontiguousarray(np.concatenate([cos, cos], 1).T)   # [128, T]
    sinT = np.ascontiguousarray(np.concatenate([-sin, sin], 1).T)  # rotate_half sign
    return cosT, sinT


def kernel(x, Wq, bq, Wk, bk, Wv, bv, Wo, bo, **_ignored):
    x = np.asarray(x, dtype=np.float32)
    Wq = np.asarray(Wq, dtype=np.float32)
    Wk = np.asarray(Wk, dtype=np.float32)
    Wv = np.asarray(Wv, dtype=np.float32)
    Wo = np.asarray(Wo, dtype=np.float32)
    bo = np.asarray(bo, dtype=np.float32)

    if "nc" not in _CACHE:
        _CACHE["nc"] = _build_nc()
    nc = _CACHE["nc"]

    cosT, sinT = _rope_tables()
    # S^T layout: mask[tk, tq] allows tk <= tq within the diagonal block
    triu = np.triu(np.ones((128, 128), dtype=bool))
    mask = np.where(triu, 0.0, -1e9).astype(np.float32)

    in_maps = []
    for c in range(8):
        b, g = c // G, c % G
        xT = x[b].T.astype(np.float16)                  # [D, T]
        # pack to SBUF layout [128, KC, T]: xp[p, e, t] = xT[e*128+p, t]
        xp = np.ascontiguousarray(
            xT.reshape(KC, 128, T).transpose(1, 0, 2))
        # wp[p, e, 0:512] = Wq[e*128+p, g cols]; 512:640 k; 640:768 v
        wq = Wq[:, g * AQ:(g + 1) * AQ].astype(np.float16)
        wk = Wk[:, g * HD:(g + 1) * HD].astype(np.float16)
        wv = Wv[:, g * HD:(g + 1) * HD].astype(np.float16)
        wcat = np.concatenate([wq, wk, wv], axis=1)     # [D, 768]
        wp = np.ascontiguousarray(
            wcat.reshape(KC, 128, 768).transpose(1, 0, 2))
        # wo[p, h, :] = Wo[g*AQ + h*128 + p, :]
        wo = np.ascontiguousarray(
            Wo[g * AQ:(g + 1) * AQ, :].astype(np.float16)
            .reshape(4, 128, D).transpose(1, 0, 2))
        in_maps.append({
            "xp": xp,
            "wp": wp,
            "wo": wo,
            "cosT": cosT,
            "sinT": sinT,
            "mask": mask,
        })

    res = run_bass_kernel_spmd(
        nc, in_maps, list(range(8)),
        trace=bool(os.environ.get("KERNEL_TRACE")),
        tmpdir=os.environ.get("KERNEL_TRACE_DIR") or None,
    )
    _CACHE["last_results"] = res

    out = np.zeros((B, T, D), dtype=np.float32)
    for b in range(B):
        acc = np.zeros((T, D), dtype=np.float32)
        for g in range(G):
            acc += res.results[b * G + g]["y"].astype(np.float32)
        out[b] = acc + bo[None, :]
    return out


# revision 5
# speedup vs baseline: 1.6953x; 1.0167x over previous
"""Grouped self-attention (GQA) Trainium2 kernel, v4.

Problem: B=2, T=2048, D=2048, 16 Q heads / 4 KV heads, head_dim=128,
full RoPE (base 1e6), causal softmax, output projection.

Sharding: 8 cores = 2 batches x 4 KV groups. Core c handles batch c//4,
kv-group c%4 (4 Q heads + 1 KV head). q/k/v projections column-sharded,
o_proj row-sharded; per-core partial outputs are summed on host.

v4 (vs v3):
- x/w DMAs per contraction chunk (fine-grained pacing; v3's [8:16]
  chunk made e>=8 matmuls wait for the whole 4MB transfer).
- phase 1b: q-projection pairs use a 4-bank PSUM pool (no bank-reuse
  stall), v transposes emitted after; during attention q passes are
  single-head on 1 bank (they are spaced far apart).
- O^T matmuls interleaved into the score-strip loop at lag 2: balances
  PE (~0.6us/strip) against ACT exp (~0.64us/strip) instead of
  stalling the PE on the exp tail each (g,h); ps_ot double-buffered so
  the normalization chain never blocks the next head.
- causal mask adds moved to gpsimd (DVE was oversubscribed in short
  groups); y DMA-out moved to the sync hardware queue (idle after
  phase 1).
"""

import os
import sys

import numpy as np

for _p in ("/opt/trn_rl_repo",):
    if _p not in sys.path and os.path.isdir(_p):
        sys.path.insert(0, _p)

import concourse.bass as bass  # noqa: E402
import concourse.mybir as mybir  # noqa: E402
import concourse.tile as tile  # noqa: E402
from concourse import bacc  # noqa: E402
from concourse.bass_utils import run_bass_kernel_spmd  # noqa: E402
from concourse.masks import make_identity  # noqa: E402

B, T, D = 2, 2048, 2048
NH, NKV, HD = 16, 4, 128
G = NKV              # kv groups == cores per batch
AQ = (NH // NKV) * HD  # attention cols per core (4 heads x 128)
KC = D // 128        # 16 contraction chunks for projections
ROPE_BASE = 1000000.0
INV_SQRT_D = 1.0 / float(np.sqrt(HD))

F32 = mybir.dt.float32
FP16 = mybir.dt.float16

PT_MAX = 512 * 13 + 768  # widest per-(g,h) P^T row (g=3): 7424


def _strips(g):
    """Score strips for tq group g (cols [512g, 512g+512)).

    Returns [(j, off, w)]: kv block j contributes group columns
    [512-w, 512); off is the strip's offset in the packed P^T buffer.
    """
    out = []
    off = 0
    for j in range(4 * g + 4):
        w = 512 - max(0, 128 * j - 512 * g)
        out.append((j, off, w))
        off += w
    return out


_CACHE = {}


def _build_nc():
    nc = bacc.Bacc(None, target_bir_lowering=False, debug=False)

    # host-packed inputs (see kernel() for layouts)
    xp_d = nc.dram_tensor("xp", [128, KC, T], FP16, kind="ExternalInput")
    wp_d = nc.dram_tensor("wp", [128, KC, 768], FP16, kind="ExternalInput")
    wo_d = nc.dram_tensor("wo", [128, 4, D], FP16, kind="ExternalInput")
    cos_d = nc.dram_tensor("cosT", [HD, T], F32, kind="ExternalInput")
    sin_d = nc.dram_tensor("sinT", [HD, T], F32, kind="ExternalInput")
    mask_d = nc.dram_tensor("mask", [128, 128], F32, kind="ExternalInput")
    y_d = nc.dram_tensor("y", [T, D], FP16, kind="ExternalOutput")

    mult = mybir.AluOpType.mult
    add = mybir.AluOpType.add
    Exp = mybir.ActivationFunctionType.Exp

    with tile.TileContext(nc) as tc:
        with (
            tc.tile_pool(name="const", bufs=1) as cpool,
            tc.tile_pool(name="qkv", bufs=1) as qkv_pool,
            tc.tile_pool(name="xw", bufs=1) as xw_pool,
            tc.tile_pool(name="ptmp", bufs=2) as tmp_pool,
        ):
            cos_sb = cpool.tile([HD, T], F32, tag="cos")
            sin_sb = cpool.tile([HD, T], F32, tag="sin")
            mask_sb = cpool.tile([128, 128], F32, tag="mask")
            id_fp = cpool.tile([128, 128], FP16, tag="idf")
            ones_sb = cpool.tile([128, 1], FP16, tag="ones")
            wo_sb = cpool.tile([128, 4, D], FP16, tag="wo")

            xt = xw_pool.tile([128, KC, T], FP16, tag="xt")
            w_all = xw_pool.tile([128, KC, 768], FP16, tag="w")
            # x/w stream per chunk, interleaved, on the sync queue;
            # everything else on the gpsimd queue.
            nc.gpsimd.dma_start(mask_sb[:], mask_d[:])
            for e in range(KC):
                nc.sync.dma_start(w_all[:, e, :], wp_d[:, e, :])
                nc.sync.dma_start(xt[:, e, :], xp_d[:, e, :])
            nc.gpsimd.dma_start(cos_sb[:], cos_d[:])
            nc.gpsimd.dma_start(sin_sb[:], sin_d[:])
            nc.gpsimd.dma_start(wo_sb[:], wo_d[:])
            make_identity(nc, id_fp[:])
            nc.gpsimd.memset(ones_sb[:], 1.0)

            qT = qkv_pool.tile([128, 4, T], FP16, tag="qT")   # [d, h, t]
            kT = qkv_pool.tile([128, T], FP16, tag="kT")      # [d, t]
            v_sb = qkv_pool.tile([128, T], FP16, tag="v")     # [tk%128, blk*128+d]
            vT_sb = qkv_pool.tile([128, T], FP16, tag="vT")   # [d, t] pre-transpose

            def rope_evict(ps, dst, tsl):
                t1 = tmp_pool.tile([128, 512], F32, tag="ropetmp")
                nc.vector.tensor_tensor(t1[:], ps[:], cos_sb[:, tsl], mult)
                nc.vector.tensor_tensor(
                    dst[0:64, :], ps[64:128, :], sin_sb[0:64, tsl], mult)
                nc.vector.tensor_tensor(
                    dst[64:128, :], ps[0:64, :], sin_sb[64:128, tsl], mult)
                nc.vector.tensor_tensor(dst[:], dst[:], t1[:], add)

            # ---- phase 1a: k+v projections, e-outer (DMA-streamed) ----
            with tc.tile_pool(name="ppkv", bufs=1, space="PSUM") as ppkv:
                psk = [ppkv.tile([128, 512], F32, tag=f"k{t}",
                                 name=f"psk{t}") for t in range(4)]
                psv = [ppkv.tile([128, 512], F32, tag=f"v{t}",
                                 name=f"psv{t}") for t in range(4)]
                for e in range(KC):
                    for tci in range(4):
                        nc.tensor.matmul(
                            psk[tci][:], w_all[:, e, 512:640],
                            xt[:, e, tci * 512:(tci + 1) * 512],
                            start=(e == 0), stop=(e == KC - 1))
                    for tci in range(4):
                        nc.tensor.matmul(
                            psv[tci][:], w_all[:, e, 640:768],
                            xt[:, e, tci * 512:(tci + 1) * 512],
                            start=(e == 0), stop=(e == KC - 1))
                for tci in range(4):
                    tsl = slice(tci * 512, (tci + 1) * 512)
                    rope_evict(psk[tci], kT[:, tsl], tsl)
                    nc.scalar.copy(vT_sb[:, tsl], psv[tci][:])

            # ---- phase 1b: q tci0 head-pairs (4 banks), v transpose ----
            def q_pass_pair(pool, ha, tci):
                tsl = slice(tci * 512, (tci + 1) * 512)
                pa = pool.tile([128, 512], F32, tag="qa",
                               name=f"q{ha}_{tci}")
                pb = pool.tile([128, 512], F32, tag="qb",
                               name=f"q{ha + 1}_{tci}")
                for e in range(KC):
                    nc.tensor.matmul(
                        pa[:], w_all[:, e, ha * 128:(ha + 1) * 128],
                        xt[:, e, tsl], start=(e == 0), stop=(e == KC - 1))
                    nc.tensor.matmul(
                        pb[:], w_all[:, e, (ha + 1) * 128:(ha + 2) * 128],
                        xt[:, e, tsl], start=(e == 0), stop=(e == KC - 1))
                rope_evict(pa, qT[:, ha, tsl], tsl)
                rope_evict(pb, qT[:, ha + 1, tsl], tsl)

            def q_pass(pool, h, tci):
                tsl = slice(tci * 512, (tci + 1) * 512)
                ps = pool.tile([128, 512], F32, tag="q1",
                               name=f"q{h}_{tci}")
                for e in range(KC):
                    nc.tensor.matmul(
                        ps[:], w_all[:, e, h * 128:(h + 1) * 128],
                        xt[:, e, tsl], start=(e == 0), stop=(e == KC - 1))
                rope_evict(ps, qT[:, h, tsl], tsl)

            with (
                tc.tile_pool(name="pq1b", bufs=2, space="PSUM") as pq1b,
                tc.tile_pool(name="pvt", bufs=2, space="PSUM") as pvt,
            ):
                q_pass_pair(pq1b, 0, 0)
                q_pass_pair(pq1b, 2, 0)
                for tci in range(4):
                    tsl = slice(tci * 512, (tci + 1) * 512)
                    pst = pvt.tile([128, 512], FP16, tag="vtr")
                    for j4 in range(4):
                        nc.tensor.transpose(
                            pst[:, j4 * 128:(j4 + 1) * 128],
                            vT_sb[:, tci * 512 + j4 * 128:
                                  tci * 512 + (j4 + 1) * 128],
                            id_fp[:],
                        )
                    nc.vector.tensor_copy(v_sb[:, tsl], pst[:])

            # ---- attention, tq-group-major, o-proj per group ----
            with (
                tc.tile_pool(name="att", bufs=2) as att_pool,
                tc.tile_pool(name="small", bufs=2) as small_pool,
                tc.tile_pool(name="yev", bufs=4) as yev_pool,
                tc.tile_pool(name="pq", bufs=1, space="PSUM") as pq_pool,
                tc.tile_pool(name="ps_st", bufs=2, space="PSUM") as st_pool,
                tc.tile_pool(name="ps_sum", bufs=1, space="PSUM") as sum_pool,
                tc.tile_pool(name="ps_ot", bufs=2, space="PSUM") as ot_pool,
                tc.tile_pool(name="ps_y", bufs=2, space="PSUM") as y_pool,
            ):
                cp = 0
                for g in range(4):
                    sl = _strips(g)
                    n = len(sl)
                    OTg = att_pool.tile([128, 4, 512], FP16, tag="OTg")
                    for h in range(4):
                        PT = att_pool.tile([128, PT_MAX], FP16, tag="PT")
                        R = att_pool.tile([128, 512], FP16, tag="R")
                        pso = ot_pool.tile([128, 512], F32, tag="OT")

                        def ot_mm(idx):
                            j, off, w = sl[idx]
                            nc.tensor.matmul(
                                pso[:, 512 - w:512],
                                v_sb[:, j * 128:(j + 1) * 128],
                                PT[:, off:off + w],
                                start=(idx == 0),
                                stop=(idx == n - 1),
                                skip_group_check=True,
                            )

                        for idx, (j, off, w) in enumerate(sl):
                            ps = st_pool.tile([128, 512], F32, tag="ST")
                            tq0 = max(512 * g, 128 * j)
                            nc.tensor.matmul(
                                ps[:, :w],
                                kT[:, j * 128:(j + 1) * 128],
                                qT[:, h, tq0:512 * g + 512],
                                start=True, stop=True,
                                skip_group_check=True)
                            if j >= 4 * g:
                                nc.vector.tensor_tensor(
                                    ps[:, :128], ps[:, :128],
                                    mask_sb[:], add)
                            nc.scalar.activation(
                                PT[:, off:off + w], ps[:, :w], Exp,
                                scale=INV_SQRT_D)
                            if idx == 0:
                                nc.vector.tensor_copy(R[:], PT[:, 0:512])
                            else:
                                nc.vector.tensor_tensor(
                                    R[:, 512 - w:], R[:, 512 - w:],
                                    PT[:, off:off + w], add)
                            if idx >= 2:
                                ot_mm(idx - 2)
                        if n >= 2:
                            ot_mm(n - 2)
                        ot_mm(n - 1)
                        # denominators: ones-matmul over R, broadcast,
                        # full-rate approx reciprocal, fold into evict
                        ps1 = sum_pool.tile([1, 512], F32, tag="SUM")
                        nc.tensor.matmul(
                            ps1[:], ones_sb[:], R[:], start=True, stop=True)
                        s_sb = small_pool.tile([1, 512], F32, tag="s")
                        nc.scalar.copy(s_sb[:], ps1[:])
                        bc = small_pool.tile([128, 512], F32, tag="bc")
                        nc.gpsimd.partition_broadcast(bc[:], s_sb[:])
                        rcp = small_pool.tile([128, 512], F32, tag="rcp")
                        nc.vector.reciprocal_approx_fast(rcp[:], bc[:])
                        nc.vector.tensor_tensor(
                            OTg[:, h, :], pso[:], rcp[:], mult)
                        # prefetch next tq group's q projection
                        if g < 3:
                            q_pass(pq_pool, h, g + 1)
                    # o-proj for this tq group
                    for tb in range(4):
                        for nci in range(4):
                            psy = y_pool.tile([128, 512], F32, tag="Y")
                            for h in range(4):
                                nc.tensor.matmul(
                                    psy[:],
                                    OTg[:, h, tb * 128:(tb + 1) * 128],
                                    wo_sb[:, h, nci * 512:(nci + 1) * 512],
                                    start=(h == 0), stop=(h == 3))
                            ysb = yev_pool.tile([128, 512], FP16, tag="ysb")
                            if cp % 2 == 0:
                                nc.scalar.copy(ysb[:], psy[:])
                            else:
                                nc.vector.tensor_copy(ysb[:], psy[:])
                            cp += 1
                            nc.sync.dma_start(
                                y_d[512 * g + tb * 128:
                                    512 * g + tb * 128 + 128,
                                    nci * 512:(nci + 1) * 512],
                                ysb[:])

    nc.compile()
    return nc


def _rope_tables():
    # match reference float32 arithmetic exactly
    pos = np.arange(T, dtype=np.float32)
    inv_freq = (1.0 / (ROPE_BASE ** (np.arange(0, HD, 2, dtype=np.float32) / HD))).astype(np.float32)
    ang = pos[:, None] * inv_freq[None, :]            # [T, 64]
    cos = np.cos(ang).astype(np.float32)
    sin = np.sin(ang).astype(np.float32)
    cosT = np.ascontiguousarray(np.concatenate([cos, cos], 1).T)   # [128, T]
    sinT = np.ascontiguousarray(np.concatenate([-sin, sin], 1).T)  # rotate_half sign
    return cosT, sinT


def kernel(x, Wq, bq, Wk, bk, Wv, bv, Wo, bo, **_ignored):
    x = np.asarray(x, dtype=np.float32)
    Wq = np.asarray(Wq, dtype=np.float32)
    Wk = np.asarray(Wk, dtype=np.float32)
    Wv = np.asarray(Wv, dtype=np.float32)
    Wo = np.asarray(Wo, dtype=np.float32)
    bo = np.asarray(bo, dtype=np.float32)

    if "nc" not in _CACHE:
        _CACHE["nc"] = _build_nc()
    nc = _CACHE["nc"]

    cosT, sinT = _rope_tables()
    # S^T layout: mask[tk, tq] allows tk <= tq within the diagonal block
    triu = np.triu(np.ones((128, 128), dtype=bool))
    mask = np.where(triu, 0.0, -1e9).astype(np.float32)

    in_maps = []
    for c in range(8):
        b, g = c // G, c % G
        xT = x[b].T.astype(np.float16)                  # [D, T]
        # pack to SBUF layout [128, KC, T]: xp[p, e, t] = xT[e*128+p, t]
        xp = np.ascontiguousarray(
            xT.reshape(KC, 128, T).transpose(1, 0, 2))
        # wp[p, e, 0:512] = Wq[e*128+p, g cols]; 512:640 k; 640:768 v
        wq = Wq[:, g * AQ:(g + 1) * AQ].astype(np.float16)
        wk = Wk[:, g * HD:(g + 1) * HD].astype(np.float16)
        wv = Wv[:, g * HD:(g + 1) * HD].astype(np.float16)
        wcat = np.concatenate([wq, wk, wv], axis=1)     # [D, 768]
        wp = np.ascontiguousarray(
            wcat.reshape(KC, 128, 768).transpose(1, 0, 2))
        # wo[p, h, :] = Wo[g*AQ + h*128 + p, :]
        wo = np.ascontiguousarray(
            Wo[g * AQ:(g + 1) * AQ, :].astype(np.float16)
            .reshape(4, 128, D).transpose(1, 0, 2))
        in_maps.append({
            "xp": xp,
            "wp": wp,
            "wo": wo,
            "cosT": cosT,
            "sinT": sinT,
            "mask": mask,
        })

    res = run_bass_kernel_spmd(
        nc, in_maps, list(range(8)),
        trace=bool(os.environ.get("KERNEL_TRACE")),
        tmpdir=os.environ.get("KERNEL_TRACE_DIR") or None,
    )
    _CACHE["last_results"] = res

    out = np.zeros((B, T, D), dtype=np.float32)
    for b in range(B):
        acc = np.zeros((T, D), dtype=np.float32)
        for g in range(G):
            acc += res.results[b * G + g]["y"].astype(np.float32)
        out[b] = acc + bo[None, :]
    return out
